# revision 11
# baseline (speedup 1.0000x reference)
"""Trainium2 Bass kernel for nn_ExogenousBasisLSTM.

Strategy (8 NeuronCores, data-parallel over batch: 16 samples/core):
- Everything in "T-layout": gate/hidden dim on SBUF partitions, batch on free.
- Per layer-step, gates^T [128, 16*16] accumulate in PSUM from:
    1 bias matmul  (lhsT = bias chunks [16,128], rhs = const delta [16,256])
    16 chunk matmuls (lhsT = packed weights [K,128], rhs = state block [K,16])
  Input-side GEMMs fold into the chunk matmuls; biases via the bias matmul,
  so rhs tiles contain only data blocks at partition-aligned bases
  (compute-engine APs must start at partition 0/32/64/96):
    enc L0 rhs [112,16]: x(0:64)    | h0(64:112)
    L1     rhs [112,16]: h1(0:48)   | 0 | h0(64:112)
    dec L0 rhs [128,16]: prev(0:48) | 0 | h0(64:112) | exog(112:128 via DMA)
- rhs tiles ping-pong so next-step input copies overlap current step.
- Gate columns reordered [i,f,o,g]: one sigmoid [128,192], one tanh [128,64].
- c stays fp32 [128,64]; matmul operands fp16 (FWL + 1 cycle/row), PSUM fp32.
- h = (sig_o*tanh c) @ Whr^T via 4 accumulating matmuls; h^T [48,16] copied
  (fp16) straight into next-step rhs tiles.
- All fp16 constants arrive in two combo DMAs (PE-consumed / DVE-consumed) so
  no compute instruction ever waits on two DMA-HW queues.
- Final theta einsums on device: broadcast-multiply + ones-reduction matmuls.
"""

from contextlib import ExitStack

import numpy as np

B, INF, OUTF, SEQ, FUT = 128, 64, 16, 512, 96
H, PROJ = 512, 48
NB = 16          # batch per core
NCORES = 8
G4 = 4 * H       # 2048

# gate reorder: old [i,f,g,o] -> new [i,f,o,g]
_PERM = np.concatenate([np.arange(512), 512 + np.arange(512),
                        1536 + np.arange(512), 1024 + np.arange(512)])

_F16 = np.float16

# column offsets inside combo tensors
_PE_SEG = [("W_e0", G4), ("W_e1", G4), ("W_d0", G4), ("W_d1", G4),
           ("WR_e0", 4 * PROJ), ("WR_e1", 4 * PROJ),
           ("WR_d0", 4 * PROJ), ("WR_d1", 4 * PROJ),
           ("sb_e0", 128), ("sb_e1", 128), ("sb_d0", 128), ("sb_d1", 128),
           ("delta", 16 * NB)]


def _pe_offsets():
    off, out = 0, {}
    for name, w in _PE_SEG:
        out[name] = (off, w)
        off += w
    return out, off


def _prep_shared(inp):
    """Pack all PE-side constants into one [128, N] fp16 array."""
    offs, total = _pe_offsets()
    combo = np.zeros((128, total), _F16)

    def put(name, arr):  # arr [rows, w] float32/float16
        o, w = offs[name]
        combo[:arr.shape[0], o:o + w] = arr.astype(_F16)

    for net, l, name in (("enc", 0, "e0"), ("enc", 1, "e1"),
                         ("dec", 0, "d0"), ("dec", 1, "d1")):
        Wih = np.asarray(inp[f"{net}_Wih{l}"])[_PERM]
        Whh = np.asarray(inp[f"{net}_Whh{l}"])[_PERM]
        bias = (np.asarray(inp[f"{net}_bih{l}"]) +
                np.asarray(inp[f"{net}_bhh{l}"]))[_PERM]
        z16 = np.zeros((16, G4), np.float32)
        if name == "e0":
            W = np.concatenate([Wih.T, Whh.T], axis=0)                 # 112
        elif name == "d0":
            W = np.concatenate([Wih[:, :PROJ].T, z16, Whh.T,
                                Wih[:, PROJ:].T], axis=0)              # 128
        else:
            W = np.concatenate([Whh.T, z16, Wih.T], axis=0)            # 112
        put(f"W_{name}", W)
        Whr = np.asarray(inp[f"{net}_Whr{l}"])
        WR = np.zeros((128, 4 * PROJ), np.float32)
        for k in range(4):
            WR[:, PROJ * k:PROJ * (k + 1)] = Whr[:, 128 * k:128 * (k + 1)].T
        put(f"WR_{name}", WR)
        put(f"sb_{name}", bias.reshape(16, 128))

    delta = np.zeros((16, 16 * NB), np.float32)
    for m in range(16):
        delta[m, NB * m:NB * (m + 1)] = 1.0
    put("delta", delta)
    return combo


def _prep_core(inp, combo, s, seq, fut):
    """in_map for the core owning batch slice [s, s+NB)."""
    ins = np.asarray(inp["insample_x_t"])[s:s + NB]      # [NB, 64, 512]
    outs = np.asarray(inp["outsample_x_t"])[s:s + NB]    # [NB, 16, 96]
    theta = np.asarray(inp["theta"])[s:s + NB]

    # dve combo: xe (64 rows, seq*NB) | fprev (48 rows, NB)
    dve = np.zeros((128, seq * NB + NB), _F16)
    dve[:INF, :seq * NB] = \
        ins[:, :, :seq].transpose(1, 2, 0).reshape(INF, seq * NB)
    dve[:PROJ, seq * NB:] = ins[:, :PROJ, SEQ - 1].T     # first_prev
    dxe = outs[:, :, :fut].transpose(1, 2, 0).reshape(OUTF, fut * NB)

    th = np.zeros((PROJ, 2 * NB), np.float32)
    th[:, :NB] = theta[:, PROJ:].T        # backcast uses theta[:, 48:]
    th[:, NB:] = theta[:, :PROJ].T        # forecast uses theta[:, :48]

    return {"combo_pe": combo, "combo_dve": np.ascontiguousarray(dve),
            "dxe": np.ascontiguousarray(dxe).astype(_F16),
            "th": np.ascontiguousarray(th)}


def _build_nc(seq, fut):
    import concourse.bacc as bacc
    import concourse.mybir as mybir
    import concourse.tile as tile

    f16 = mybir.dt.float16
    f32 = mybir.dt.float32
    f32r = mybir.dt.float32r
    AF = mybir.ActivationFunctionType

    offs, pe_total = _pe_offsets()

    nc = bacc.Bacc()
    d_pe = nc.declare_dram_parameter("combo_pe", [128, pe_total], f16,
                                     isOutput=False)
    d_dve = nc.declare_dram_parameter("combo_dve", [128, seq * NB + NB], f16,
                                      isOutput=False)
    d_dxe = nc.declare_dram_parameter("dxe", [OUTF, fut * NB], f16,
                                      isOutput=False)
    d_th = nc.declare_dram_parameter("th", [PROJ, 2 * NB], f32, isOutput=False)
    d_bc = nc.declare_dram_parameter("bc_t", [1, seq * NB], f32, isOutput=True)
    d_fc = nc.declare_dram_parameter("fc_t", [1, fut * NB], f32, isOutput=True)

    with tile.TileContext(nc) as tc, ExitStack() as ctx:
        const = ctx.enter_context(tc.tile_pool(name="const", bufs=1))
        state = ctx.enter_context(tc.tile_pool(name="state", bufs=1))
        work = ctx.enter_context(tc.tile_pool(name="work", bufs=3))
        pgates = ctx.enter_context(tc.tile_pool(name="pgates", bufs=2, space="PSUM"))
        ph = ctx.enter_context(tc.tile_pool(name="ph", bufs=2, space="PSUM"))
        pout = ctx.enter_context(tc.tile_pool(name="pout", bufs=2, space="PSUM"))

        pe = const.tile([128, pe_total], f16, tag="pe")
        nc.sync.dma_start(pe[:], d_pe[:])
        dve = const.tile([128, seq * NB + NB], f16, tag="dve")
        nc.sync.dma_start(dve[:], d_dve[:])
        th = const.tile([PROJ, 2 * NB], f32, tag="th")
        nc.sync.dma_start(th[:], d_th[:])

        def cseg(name, rows=128):
            o, w = offs[name]
            return pe[0:rows, o:o + w]

        xe = dve[0:INF, 0:seq * NB]
        fprev = dve[0:PROJ, seq * NB:seq * NB + NB]

        rhs0, rhs1, rhs0d = [], [], []
        for i in range(2):
            t = state.tile([112, NB], f16, tag=f"rhs0_{i}")
            nc.vector.memset(t[:], 0.0)
            rhs0.append(t)
            t = state.tile([112, NB], f16, tag=f"rhs1_{i}")
            nc.vector.memset(t[:], 0.0)
            rhs1.append(t)
            t = state.tile([128, NB], f16, tag=f"rhs0d_{i}")
            nc.vector.memset(t[:], 0.0)
            rhs0d.append(t)

        c0 = state.tile([128, 4 * NB], f32, tag="c0")
        c1 = state.tile([128, 4 * NB], f32, tag="c1")
        nc.vector.memset(c0[:], 0.0)
        nc.vector.memset(c1[:], 0.0)
        hs1 = state.tile([PROJ, seq * NB], f32, tag="hs1")
        preds = state.tile([PROJ, fut * NB], f32, tag="preds")

        delta = cseg("delta", 16)

        def layer_step(wname, rhs, h16_dsts, c, h32_dsts):
            Wn = cseg(f"W_{wname}", rhs.shape[0])
            WRn = cseg(f"WR_{wname}")
            sbias = cseg(f"sb_{wname}", 16)
            g = pgates.tile([128, 16 * NB], f32, tag="gates")
            nc.tensor.matmul(g[:], sbias, delta, start=True, stop=False)
            for m in range(16):
                nc.tensor.matmul(g[:, NB * m:NB * (m + 1)],
                                 Wn[:, 128 * m:128 * (m + 1)], rhs[:],
                                 start=False, stop=(m == 15))
            sig = work.tile([128, 12 * NB], f32, tag="sig")
            nc.scalar.activation(sig[:], g[:, :12 * NB], AF.Sigmoid)
            tg = work.tile([128, 4 * NB], f32, tag="tg")
            nc.scalar.activation(tg[:], g[:, 12 * NB:], AF.Tanh)
            m1 = work.tile([128, 4 * NB], f32, tag="m1")
            nc.vector.tensor_mul(m1[:], sig[:, :4 * NB], tg[:])
            nc.vector.tensor_mul(c[:], c[:], sig[:, 4 * NB:8 * NB])
            nc.vector.tensor_add(c[:], c[:], m1[:])
            tcc = work.tile([128, 4 * NB], f32, tag="tcc")
            nc.scalar.activation(tcc[:], c[:], AF.Tanh)
            u = work.tile([128, 4 * NB], f16, tag="u")
            nc.vector.tensor_mul(u[:], sig[:, 8 * NB:], tcc[:])
            hp = ph.tile([PROJ, NB], f32, tag="hp")
            for k in range(4):
                nc.tensor.matmul(hp[:], WRn[:, PROJ * k:PROJ * (k + 1)],
                                 u[:, NB * k:NB * (k + 1)],
                                 start=(k == 0), stop=(k == 3))
            for dst in h16_dsts:
                nc.scalar.copy(dst, hp[:])
            for dst in h32_dsts:
                nc.scalar.copy(dst, hp[:])

        # ---------------- encoder ----------------
        nc.vector.tensor_copy(rhs0[0][0:INF, :], xe[:, 0:NB])
        for t in range(seq):
            b, nb = t % 2, (t + 1) % 2
            if t + 1 < seq:
                nc.vector.tensor_copy(rhs0[nb][0:INF, :],
                                      xe[:, NB * (t + 1):NB * (t + 2)])
            layer_step("e0", rhs0[b],
                       [rhs0[nb][64:112, :], rhs1[b][64:112, :]], c0, [])
            layer_step("e1", rhs1[b],
                       [rhs1[nb][0:48, :]], c1,
                       [hs1[:, NB * t:NB * (t + 1)]])

        # ---------------- decoder ----------------
        nc.vector.tensor_copy(rhs0d[0][0:48, :], fprev)
        nc.scalar.copy(rhs0d[0][64:112, :], rhs0[seq % 2][64:112, :])
        nc.sync.dma_start(rhs0d[0][112:128, :], d_dxe[:, 0:NB])
        for t in range(fut):
            b, nb = t % 2, (t + 1) % 2
            if t + 1 < fut:
                nc.sync.dma_start(rhs0d[nb][112:128, :],
                                  d_dxe[:, NB * (t + 1):NB * (t + 2)])
            layer_step("d0", rhs0d[b],
                       [rhs0d[nb][64:112, :], rhs1[b][64:112, :]], c0, [])
            layer_step("d1", rhs1[b],
                       [rhs1[nb][0:48, :], rhs0d[nb][0:48, :]], c1,
                       [preds[:, NB * t:NB * (t + 1)]])

        # ---------------- theta einsums ----------------
        ones48 = const.tile([PROJ, 1], f32, tag="ones48")
        nc.vector.memset(ones48[:], 1.0)

        def reduce_out(src, thv, n, out_sb, tag):
            # out[0, j] = sum_p thv[p, j%NB] * src[p, j]
            mulbuf = state.tile([PROJ, n], f32, tag=f"mul_{tag}")
            nt = n // NB
            nc.vector.tensor_mul(
                mulbuf[:].rearrange("p (t b) -> p t b", b=NB),
                src[:].rearrange("p (t b) -> p t b", b=NB),
                thv[:, None, :].broadcast_to([PROJ, nt, NB]))
            for j in range(0, n, 512):
                w = min(512, n - j)
                ps = pout.tile([1, 512], f32, tag="pout")
                nc.tensor.matmul(ps[:, :w], ones48[:], mulbuf[:, j:j + w],
                                 start=True, stop=True)
                nc.scalar.copy(out_sb[:, j:j + w], ps[:, :w])

        bct = state.tile([1, seq * NB], f32, tag="bct")
        fct = state.tile([1, fut * NB], f32, tag="fct")
        reduce_out(hs1, th[:, 0:NB], seq * NB, bct, "bc")
        reduce_out(preds, th[:, NB:2 * NB], fut * NB, fct, "fc")
        nc.sync.dma_start(d_bc[:], bct[:])
        nc.sync.dma_start(d_fc[:], fct[:])

    nc.finalize()
    return nc


_NC_CACHE = {}


def _get_nc(seq, fut):
    key = (seq, fut)
    if key not in _NC_CACHE:
        _NC_CACHE[key] = _build_nc(seq, fut)
    return _NC_CACHE[key]


def kernel(theta, insample_x_t, outsample_x_t, _seq=SEQ, _fut=FUT, **weights):
    from concourse.bass_utils import run_bass_kernel_spmd

    inp = dict(weights)
    inp["theta"] = theta
    inp["insample_x_t"] = insample_x_t
    inp["outsample_x_t"] = outsample_x_t

    nc = _get_nc(_seq, _fut)
    combo = _prep_shared(inp)
    in_maps = [_prep_core(inp, combo, NB * i, _seq, _fut) for i in range(NCORES)]
    res = run_bass_kernel_spmd(nc, in_maps, list(range(NCORES)))

    bc = np.empty((B, _seq), np.float32)
    fc = np.empty((B, _fut), np.float32)
    for i in range(NCORES):
        r = res.results[i]
        bc[NB * i:NB * (i + 1)] = r["bc_t"].reshape(_seq, NB).T
        fc[NB * i:NB * (i + 1)] = r["fc_t"].reshape(_fut, NB).T
    return bc, fc


def bench(inp, _seq=SEQ, _fut=FUT):
    """Profiled run; returns HW exec time in ns (max across cores)."""
    from concourse.bass_utils import run_bass_kernel_spmd

    nc = _get_nc(_seq, _fut)
    combo = _prep_shared(inp)
    in_maps = [_prep_core(inp, combo, NB * i, _seq, _fut) for i in range(NCORES)]
    res = run_bass_kernel_spmd(nc, in_maps, list(range(NCORES)), trace=True)
    t = res.exec_time_ns
    if t is None:
        t = res.mean_exec_time_ns
    return t


# revision 12
# speedup vs baseline: 1.0991x; 1.0991x over previous
"""Trainium2 Bass kernel for nn_ExogenousBasisLSTM.

Strategy (8 NeuronCores, data-parallel over batch: 16 samples/core):
- Everything in "T-layout": gate/hidden dim on SBUF partitions, batch on free.
- Per layer-step, gates^T [128, 16*16] accumulate in PSUM from:
    1 bias matmul  (lhsT = bias chunks [16,128], rhs = const delta [16,256])
    16 chunk matmuls (lhsT = packed weights [K,128], rhs = state block [K,16])
  Input-side GEMMs fold into the chunk matmuls; biases via the bias matmul,
  so rhs tiles contain only data blocks at partition-aligned bases
  (compute-engine APs must start at partition 0/32/64/96):
    enc L0 rhs [112,16]: x(0:64)    | h0(64:112)
    L1     rhs [112,16]: h1(0:48)   | 0 | h0(64:112)
    dec L0 rhs [128,16]: prev(0:48) | 0 | h0(64:112) | exog(112:128 via DMA)
- rhs tiles ping-pong so next-step input copies overlap current step.
- Gate columns reordered [i,f,o,g]: one sigmoid [128,192], one tanh [128,64].
- c stays fp32 [128,64]; matmul operands fp16 (FWL + 1 cycle/row), PSUM fp32.
- h = (sig_o*tanh c) @ Whr^T via 4 accumulating matmuls; h^T [48,16] copied
  (fp16) straight into next-step rhs tiles.
- All fp16 constants arrive in two combo DMAs (PE-consumed / DVE-consumed) so
  no compute instruction ever waits on two DMA-HW queues.
- Final theta einsums on device: broadcast-multiply + ones-reduction matmuls.
"""

from contextlib import ExitStack

import numpy as np

B, INF, OUTF, SEQ, FUT = 128, 64, 16, 512, 96
H, PROJ = 512, 48
NB = 16          # batch per core
NCORES = 8
G4 = 4 * H       # 2048

# gate reorder: old [i,f,g,o] -> new [i,f,o,g]
_PERM = np.concatenate([np.arange(512), 512 + np.arange(512),
                        1536 + np.arange(512), 1024 + np.arange(512)])

_F16 = np.float16

# column offsets inside combo tensors
_PE_SEG = [("W_e0", G4), ("W_e1", G4), ("W_d0", G4), ("W_d1", G4),
           ("WR_e0", 4 * PROJ), ("WR_e1", 4 * PROJ),
           ("WR_d0", 4 * PROJ), ("WR_d1", 4 * PROJ),
           ("sb_e0", 128), ("sb_e1", 128), ("sb_d0", 128), ("sb_d1", 128),
           ("delta", 16 * NB)]


def _pe_offsets():
    off, out = 0, {}
    for name, w in _PE_SEG:
        out[name] = (off, w)
        off += w
    return out, off


def _prep_shared(inp):
    """Pack all PE-side constants into one [128, N] fp16 array."""
    offs, total = _pe_offsets()
    combo = np.zeros((128, total), _F16)

    def put(name, arr):  # arr [rows, w] float32/float16
        o, w = offs[name]
        combo[:arr.shape[0], o:o + w] = arr.astype(_F16)

    for net, l, name in (("enc", 0, "e0"), ("enc", 1, "e1"),
                         ("dec", 0, "d0"), ("dec", 1, "d1")):
        Wih = np.asarray(inp[f"{net}_Wih{l}"])[_PERM]
        Whh = np.asarray(inp[f"{net}_Whh{l}"])[_PERM]
        bias = (np.asarray(inp[f"{net}_bih{l}"]) +
                np.asarray(inp[f"{net}_bhh{l}"]))[_PERM]
        z16 = np.zeros((16, G4), np.float32)
        if name == "e0":
            W = np.concatenate([Wih.T, Whh.T], axis=0)                 # 112
        elif name == "d0":
            W = np.concatenate([Wih[:, :PROJ].T, z16, Whh.T,
                                Wih[:, PROJ:].T], axis=0)              # 128
        else:
            W = np.concatenate([Whh.T, z16, Wih.T], axis=0)            # 112
        put(f"W_{name}", W)
        Whr = np.asarray(inp[f"{net}_Whr{l}"])
        WR = np.zeros((128, 4 * PROJ), np.float32)
        for k in range(4):
            WR[:, PROJ * k:PROJ * (k + 1)] = Whr[:, 128 * k:128 * (k + 1)].T
        put(f"WR_{name}", WR)
        put(f"sb_{name}", bias.reshape(16, 128))

    delta = np.zeros((16, 16 * NB), np.float32)
    for m in range(16):
        delta[m, NB * m:NB * (m + 1)] = 1.0
    put("delta", delta)
    return combo


def _prep_core(inp, combo, s, seq, fut):
    """in_map for the core owning batch slice [s, s+NB)."""
    ins = np.asarray(inp["insample_x_t"])[s:s + NB]      # [NB, 64, 512]
    outs = np.asarray(inp["outsample_x_t"])[s:s + NB]    # [NB, 16, 96]
    theta = np.asarray(inp["theta"])[s:s + NB]

    # dve combo: xe (64r, seq*NB) | fprev (48r, NB) | thb,thf (48r, 2*NB)
    dve = np.zeros((128, seq * NB + 3 * NB), _F16)
    dve[:INF, :seq * NB] = \
        ins[:, :, :seq].transpose(1, 2, 0).reshape(INF, seq * NB)
    dve[:PROJ, seq * NB:seq * NB + NB] = ins[:, :PROJ, SEQ - 1].T  # first_prev
    dve[:PROJ, seq * NB + NB:seq * NB + 2 * NB] = theta[:, PROJ:].T
    dve[:PROJ, seq * NB + 2 * NB:] = theta[:, :PROJ].T
    dxe = outs[:, :, :fut].transpose(1, 2, 0).reshape(OUTF, fut * NB)

    return {"combo_pe": combo, "combo_dve": np.ascontiguousarray(dve),
            "dxe": np.ascontiguousarray(dxe).astype(_F16)}


def _build_nc(seq, fut):
    import concourse.bacc as bacc
    import concourse.mybir as mybir
    import concourse.tile as tile

    f16 = mybir.dt.float16
    f32 = mybir.dt.float32
    f32r = mybir.dt.float32r
    AF = mybir.ActivationFunctionType

    offs, pe_total = _pe_offsets()

    nc = bacc.Bacc()
    d_pe = nc.declare_dram_parameter("combo_pe", [128, pe_total], f16,
                                     isOutput=False)
    d_dve = nc.declare_dram_parameter("combo_dve", [128, seq * NB + 3 * NB], f16,
                                      isOutput=False)
    d_dxe = nc.declare_dram_parameter("dxe", [OUTF, fut * NB], f16,
                                      isOutput=False)
    d_bc = nc.declare_dram_parameter("bc_t", [1, seq * NB], f32, isOutput=True)
    d_fc = nc.declare_dram_parameter("fc_t", [1, fut * NB], f32, isOutput=True)

    with tile.TileContext(nc) as tc, ExitStack() as ctx:
        const = ctx.enter_context(tc.tile_pool(name="const", bufs=1))
        state = ctx.enter_context(tc.tile_pool(name="state", bufs=1))
        work = ctx.enter_context(tc.tile_pool(name="work", bufs=3))
        pgates = ctx.enter_context(tc.tile_pool(name="pgates", bufs=2, space="PSUM"))
        ph = ctx.enter_context(tc.tile_pool(name="ph", bufs=2, space="PSUM"))
        pout = ctx.enter_context(tc.tile_pool(name="pout", bufs=2, space="PSUM"))

        pe = const.tile([128, pe_total], f16, tag="pe")
        nc.sync.dma_start(pe[:], d_pe[:])
        dve = const.tile([128, seq * NB + 3 * NB], f16, tag="dve")
        nc.sync.dma_start(dve[:], d_dve[:])

        def cseg(name, rows=128):
            o, w = offs[name]
            return pe[0:rows, o:o + w]

        xe = dve[0:INF, 0:seq * NB]
        fprev = dve[0:PROJ, seq * NB:seq * NB + NB]
        thb = dve[0:PROJ, seq * NB + NB:seq * NB + 2 * NB]
        thf = dve[0:PROJ, seq * NB + 2 * NB:seq * NB + 3 * NB]

        rhs0, rhs1, rhs0d = [], [], []
        for i in range(2):
            t = state.tile([112, NB], f16, tag=f"rhs0_{i}")
            nc.vector.memset(t[:], 0.0)
            rhs0.append(t)
            t = state.tile([112, NB], f16, tag=f"rhs1_{i}")
            nc.vector.memset(t[:], 0.0)
            rhs1.append(t)
            t = state.tile([128, NB], f16, tag=f"rhs0d_{i}")
            nc.vector.memset(t[:], 0.0)
            rhs0d.append(t)

        c0 = state.tile([128, 4 * NB], f32, tag="c0")
        c1 = state.tile([128, 4 * NB], f32, tag="c1")
        nc.vector.memset(c0[:], 0.0)
        nc.vector.memset(c1[:], 0.0)
        hs1 = state.tile([PROJ, seq * NB], f16, tag="hs1")
        preds = state.tile([PROJ, fut * NB], f16, tag="preds")

        delta = cseg("delta", 16)

        def layer_step(wname, rhs, c, primary, gp_dsts):
            Wn = cseg(f"W_{wname}", rhs.shape[0])
            WRn = cseg(f"WR_{wname}")
            sbias = cseg(f"sb_{wname}", 16)
            g = pgates.tile([128, 16 * NB], f32, tag="gates")
            nc.tensor.matmul(g[:], sbias, delta, start=True, stop=False)
            for m in range(16):
                nc.tensor.matmul(g[:, NB * m:NB * (m + 1)],
                                 Wn[:, 128 * m:128 * (m + 1)], rhs[:],
                                 start=False, stop=(m == 15))
            sig = work.tile([128, 12 * NB], f32, tag="sig")
            nc.scalar.activation(sig[:], g[:, :12 * NB], AF.Sigmoid)
            tg = work.tile([128, 4 * NB], f32, tag="tg")
            nc.scalar.activation(tg[:], g[:, 12 * NB:], AF.Tanh)
            m1 = work.tile([128, 4 * NB], f32, tag="m1")
            nc.vector.tensor_mul(m1[:], sig[:, :4 * NB], tg[:])
            nc.vector.tensor_mul(c[:], c[:], sig[:, 4 * NB:8 * NB])
            nc.vector.tensor_add(c[:], c[:], m1[:])
            tcc = work.tile([128, 4 * NB], f32, tag="tcc")
            nc.scalar.activation(tcc[:], c[:], AF.Tanh)
            u = work.tile([128, 4 * NB], f16, tag="u")
            nc.vector.tensor_mul(u[:], sig[:, 8 * NB:], tcc[:])
            hp = ph.tile([PROJ, NB], f32, tag="hp")
            for k in range(4):
                nc.tensor.matmul(hp[:], WRn[:, PROJ * k:PROJ * (k + 1)],
                                 u[:, NB * k:NB * (k + 1)],
                                 start=(k == 0), stop=(k == 3))
            nc.vector.tensor_copy(primary, hp[:])
            for dst in gp_dsts:
                nc.gpsimd.tensor_copy(dst, primary)

        # ---------------- encoder ----------------
        nc.gpsimd.tensor_copy(rhs0[0][0:INF, :], xe[:, 0:NB])
        for t in range(seq):
            b, nb = t % 2, (t + 1) % 2
            if t + 1 < seq:
                nc.gpsimd.tensor_copy(rhs0[nb][0:INF, :],
                                      xe[:, NB * (t + 1):NB * (t + 2)])
            layer_step("e0", rhs0[b], c0, rhs1[b][64:112, :],
                       [rhs0[nb][64:112, :]])
            layer_step("e1", rhs1[b], c1, rhs1[nb][0:48, :],
                       [hs1[:, NB * t:NB * (t + 1)]])

        # ---------------- decoder ----------------
        nc.gpsimd.tensor_copy(rhs0d[0][0:48, :], fprev)
        nc.gpsimd.tensor_copy(rhs0d[0][64:112, :], rhs0[seq % 2][64:112, :])
        nc.sync.dma_start(rhs0d[0][112:128, :], d_dxe[:, 0:NB])
        for t in range(fut):
            b, nb = t % 2, (t + 1) % 2
            if t + 1 < fut:
                nc.sync.dma_start(rhs0d[nb][112:128, :],
                                  d_dxe[:, NB * (t + 1):NB * (t + 2)])
            layer_step("d0", rhs0d[b], c0, rhs1[b][64:112, :],
                       [rhs0d[nb][64:112, :]])
            layer_step("d1", rhs1[b], c1, rhs1[nb][0:48, :],
                       [rhs0d[nb][0:48, :], preds[:, NB * t:NB * (t + 1)]])

        # ---------------- theta einsums ----------------
        ones48 = const.tile([PROJ, 1], f32, tag="ones48")
        nc.vector.memset(ones48[:], 1.0)

        def reduce_out(src, thv, n, out_sb, tag):
            # out[0, j] = sum_p thv[p, j%NB] * src[p, j]
            mulbuf = state.tile([PROJ, n], f32, tag=f"mul_{tag}")
            nt = n // NB
            nc.vector.tensor_mul(
                mulbuf[:].rearrange("p (t b) -> p t b", b=NB),
                src[:].rearrange("p (t b) -> p t b", b=NB),
                thv[:, None, :].broadcast_to([PROJ, nt, NB]))
            for j in range(0, n, 512):
                w = min(512, n - j)
                ps = pout.tile([1, 512], f32, tag="pout")
                nc.tensor.matmul(ps[:, :w], ones48[:], mulbuf[:, j:j + w],
                                 start=True, stop=True)
                nc.vector.tensor_copy(out_sb[:, j:j + w], ps[:, :w])

        bct = state.tile([1, seq * NB], f32, tag="bct")
        fct = state.tile([1, fut * NB], f32, tag="fct")
        reduce_out(hs1, thb, seq * NB, bct, "bc")
        reduce_out(preds, thf, fut * NB, fct, "fc")
        nc.sync.dma_start(d_bc[:], bct[:])
        nc.sync.dma_start(d_fc[:], fct[:])

    nc.finalize()
    return nc


_NC_CACHE = {}


def _get_nc(seq, fut):
    key = (seq, fut)
    if key not in _NC_CACHE:
        _NC_CACHE[key] = _build_nc(seq, fut)
    return _NC_CACHE[key]


def kernel(theta, insample_x_t, outsample_x_t, _seq=SEQ, _fut=FUT, **weights):
    from concourse.bass_utils import run_bass_kernel_spmd

    inp = dict(weights)
    inp["theta"] = theta
    inp["insample_x_t"] = insample_x_t
    inp["outsample_x_t"] = outsample_x_t

    nc = _get_nc(_seq, _fut)
    combo = _prep_shared(inp)
    in_maps = [_prep_core(inp, combo, NB * i, _seq, _fut) for i in range(NCORES)]
    res = run_bass_kernel_spmd(nc, in_maps, list(range(NCORES)))

    bc = np.empty((B, _seq), np.float32)
    fc = np.empty((B, _fut), np.float32)
    for i in range(NCORES):
        r = res.results[i]
        bc[NB * i:NB * (i + 1)] = r["bc_t"].reshape(_seq, NB).T
        fc[NB * i:NB * (i + 1)] = r["fc_t"].reshape(_fut, NB).T
    return bc, fc


def bench(inp, _seq=SEQ, _fut=FUT):
    """Profiled run; returns HW exec time in ns (max across cores)."""
    from concourse.bass_utils import run_bass_kernel_spmd

    nc = _get_nc(_seq, _fut)
    combo = _prep_shared(inp)
    in_maps = [_prep_core(inp, combo, NB * i, _seq, _fut) for i in range(NCORES)]
    res = run_bass_kernel_spmd(nc, in_maps, list(range(NCORES)), trace=True)
    t = res.exec_time_ns
    if t is None:
        t = res.mean_exec_time_ns
    return t


# revision 13
# speedup vs baseline: 1.1154x; 1.0149x over previous
"""Trainium2 Bass kernel for nn_ExogenousBasisLSTM.

Strategy (8 NeuronCores, data-parallel over batch: 16 samples/core):
- Everything in "T-layout": gate/hidden dim on SBUF partitions, batch on free.
- Per layer-step, gates^T [128, 16*16] accumulate in PSUM from:
    1 bias matmul  (lhsT = bias chunks [16,128], rhs = const delta [16,256])
    16 chunk matmuls (lhsT = packed weights [K,128], rhs = state block [K,16])
  Input-side GEMMs fold into the chunk matmuls; biases via the bias matmul,
  so rhs tiles contain only data blocks at partition-aligned bases
  (compute-engine APs must start at partition 0/32/64/96):
    enc L0 rhs [112,16]: x(0:64)    | h0(64:112)
    L1     rhs [112,16]: h1(0:48)   | 0 | h0(64:112)
    dec L0 rhs [128,16]: prev(0:48) | 0 | h0(64:112) | exog(112:128 via DMA)
- rhs tiles ping-pong so next-step input copies overlap current step.
- Gate columns reordered [i,f,o,g]: one sigmoid [128,192], one tanh [128,64].
- c stays fp32 [128,64]; matmul operands fp16 (FWL + 1 cycle/row), PSUM fp32.
- h = (sig_o*tanh c) @ Whr^T via 4 accumulating matmuls; h^T [48,16] copied
  (fp16) straight into next-step rhs tiles.
- All fp16 constants arrive in two combo DMAs (PE-consumed / DVE-consumed) so
  no compute instruction ever waits on two DMA-HW queues.
- Final theta einsums on device: broadcast-multiply + ones-reduction matmuls.
"""

from contextlib import ExitStack

import numpy as np

B, INF, OUTF, SEQ, FUT = 128, 64, 16, 512, 96
H, PROJ = 512, 48
NB = 16          # batch per core
NCORES = 8
G4 = 4 * H       # 2048

# gate reorder: old [i,f,g,o] -> new [i,f,o,g]
_PERM = np.concatenate([np.arange(512), 512 + np.arange(512),
                        1536 + np.arange(512), 1024 + np.arange(512)])

_F16 = np.float16

# column offsets inside combo tensors
_PE_SEG = [("W_e0", G4), ("W_e1", G4), ("W_d0", G4), ("W_d1", G4),
           ("WR_e0", 4 * PROJ), ("WR_e1", 4 * PROJ),
           ("WR_d0", 4 * PROJ), ("WR_d1", 4 * PROJ),
           ("sb_e0", 128), ("sb_e1", 128), ("sb_d0", 128), ("sb_d1", 128),
           ("delta", 16 * NB)]


def _pe_offsets():
    off, out = 0, {}
    for name, w in _PE_SEG:
        out[name] = (off, w)
        off += w
    return out, off


def _prep_shared(inp):
    """Pack all PE-side constants into one [128, N] fp16 array."""
    offs, total = _pe_offsets()
    combo = np.zeros((128, total), _F16)

    def put(name, arr):  # arr [rows, w] float32/float16
        o, w = offs[name]
        combo[:arr.shape[0], o:o + w] = arr.astype(_F16)

    for net, l, name in (("enc", 0, "e0"), ("enc", 1, "e1"),
                         ("dec", 0, "d0"), ("dec", 1, "d1")):
        Wih = np.asarray(inp[f"{net}_Wih{l}"])[_PERM]
        Whh = np.asarray(inp[f"{net}_Whh{l}"])[_PERM]
        bias = (np.asarray(inp[f"{net}_bih{l}"]) +
                np.asarray(inp[f"{net}_bhh{l}"]))[_PERM]
        z16 = np.zeros((16, G4), np.float32)
        if name == "e0":
            W = np.concatenate([Wih.T, Whh.T], axis=0)                 # 112
        elif name == "d0":
            W = np.concatenate([Wih[:, :PROJ].T, z16, Whh.T,
                                Wih[:, PROJ:].T], axis=0)              # 128
        else:
            W = np.concatenate([Whh.T, z16, Wih.T], axis=0)            # 112
        put(f"W_{name}", W)
        Whr = np.asarray(inp[f"{net}_Whr{l}"])
        WR = np.zeros((128, 4 * PROJ), np.float32)
        for k in range(4):
            WR[:, PROJ * k:PROJ * (k + 1)] = Whr[:, 128 * k:128 * (k + 1)].T
        put(f"WR_{name}", WR)
        put(f"sb_{name}", bias.reshape(16, 128))

    delta = np.zeros((16, 16 * NB), np.float32)
    for m in range(16):
        delta[m, NB * m:NB * (m + 1)] = 1.0
    put("delta", delta)
    return combo


def _prep_core(inp, combo, s, seq, fut):
    """in_map for the core owning batch slice [s, s+NB)."""
    ins = np.asarray(inp["insample_x_t"])[s:s + NB]      # [NB, 64, 512]
    outs = np.asarray(inp["outsample_x_t"])[s:s + NB]    # [NB, 16, 96]
    theta = np.asarray(inp["theta"])[s:s + NB]

    # dve combo: xe (64r, seq*NB) | fprev (48r, NB) | thb,thf (48r, 2*NB)
    dve = np.zeros((128, seq * NB + 3 * NB), _F16)
    dve[:INF, :seq * NB] = \
        ins[:, :, :seq].transpose(1, 2, 0).reshape(INF, seq * NB)
    dve[:PROJ, seq * NB:seq * NB + NB] = ins[:, :PROJ, SEQ - 1].T  # first_prev
    dve[:PROJ, seq * NB + NB:seq * NB + 2 * NB] = theta[:, PROJ:].T
    dve[:PROJ, seq * NB + 2 * NB:] = theta[:, :PROJ].T
    dxe = outs[:, :, :fut].transpose(1, 2, 0).reshape(OUTF, fut * NB)

    return {"combo_pe": combo, "combo_dve": np.ascontiguousarray(dve),
            "dxe": np.ascontiguousarray(dxe).astype(_F16)}


def _build_nc(seq, fut):
    import concourse.bacc as bacc
    import concourse.mybir as mybir
    import concourse.tile as tile

    f16 = mybir.dt.float16
    f32 = mybir.dt.float32
    f32r = mybir.dt.float32r
    AF = mybir.ActivationFunctionType

    offs, pe_total = _pe_offsets()

    nc = bacc.Bacc()
    d_pe = nc.declare_dram_parameter("combo_pe", [128, pe_total], f16,
                                     isOutput=False)
    d_dve = nc.declare_dram_parameter("combo_dve", [128, seq * NB + 3 * NB], f16,
                                      isOutput=False)
    d_dxe = nc.declare_dram_parameter("dxe", [OUTF, fut * NB], f16,
                                      isOutput=False)
    d_bc = nc.declare_dram_parameter("bc_t", [1, seq * NB], f32, isOutput=True)
    d_fc = nc.declare_dram_parameter("fc_t", [1, fut * NB], f32, isOutput=True)

    with tile.TileContext(nc) as tc, ExitStack() as ctx:
        const = ctx.enter_context(tc.tile_pool(name="const", bufs=1))
        state = ctx.enter_context(tc.tile_pool(name="state", bufs=1))
        work = ctx.enter_context(tc.tile_pool(name="work", bufs=4))
        pgates = ctx.enter_context(tc.tile_pool(name="pgates", bufs=3, space="PSUM"))
        ph = ctx.enter_context(tc.tile_pool(name="ph", bufs=2, space="PSUM"))
        pout = ctx.enter_context(tc.tile_pool(name="pout", bufs=2, space="PSUM"))

        pe = const.tile([128, pe_total], f16, tag="pe")
        nc.sync.dma_start(pe[:], d_pe[:])
        dve = const.tile([128, seq * NB + 3 * NB], f16, tag="dve")
        nc.sync.dma_start(dve[:], d_dve[:])

        def cseg(name, rows=128):
            o, w = offs[name]
            return pe[0:rows, o:o + w]

        xe = dve[0:INF, 0:seq * NB]
        fprev = dve[0:PROJ, seq * NB:seq * NB + NB]
        thb = dve[0:PROJ, seq * NB + NB:seq * NB + 2 * NB]
        thf = dve[0:PROJ, seq * NB + 2 * NB:seq * NB + 3 * NB]

        rhs0, rhs1, rhs0d = [], [], []
        for i in range(2):
            t = state.tile([112, NB], f16, tag=f"rhs0_{i}")
            nc.vector.memset(t[:], 0.0)
            rhs0.append(t)
            t = state.tile([112, NB], f16, tag=f"rhs1_{i}")
            nc.vector.memset(t[:], 0.0)
            rhs1.append(t)
            t = state.tile([128, NB], f16, tag=f"rhs0d_{i}")
            nc.vector.memset(t[:], 0.0)
            rhs0d.append(t)

        c0 = state.tile([128, 4 * NB], f32, tag="c0")
        c1 = state.tile([128, 4 * NB], f32, tag="c1")
        nc.vector.memset(c0[:], 0.0)
        nc.vector.memset(c1[:], 0.0)
        hs1 = state.tile([PROJ, seq * NB], f16, tag="hs1")
        preds = state.tile([PROJ, fut * NB], f16, tag="preds")

        delta = cseg("delta", 16)

        def layer_step(wname, rhs, c, primary, gp_dsts):
            Wn = cseg(f"W_{wname}", rhs.shape[0])
            WRn = cseg(f"WR_{wname}")
            sbias = cseg(f"sb_{wname}", 16)
            g = pgates.tile([128, 16 * NB], f32, tag="gates")
            nc.tensor.matmul(g[:], sbias, delta, start=True, stop=False)
            for m in range(16):
                nc.tensor.matmul(g[:, NB * m:NB * (m + 1)],
                                 Wn[:, 128 * m:128 * (m + 1)], rhs[:],
                                 start=False, stop=(m == 15))
            sig = work.tile([128, 12 * NB], f32, tag="sig")
            nc.scalar.activation(sig[:, :8 * NB], g[:, :8 * NB], AF.Sigmoid)
            tg = work.tile([128, 4 * NB], f32, tag="tg")
            nc.scalar.activation(tg[:], g[:, 12 * NB:], AF.Tanh)
            nc.scalar.activation(sig[:, 8 * NB:], g[:, 8 * NB:12 * NB],
                                 AF.Sigmoid)
            nc.vector.tensor_mul(c[:], c[:], sig[:, 4 * NB:8 * NB])
            m1 = work.tile([128, 4 * NB], f32, tag="m1")
            nc.vector.tensor_mul(m1[:], sig[:, :4 * NB], tg[:])
            nc.vector.tensor_add(c[:], c[:], m1[:])
            tcc = work.tile([128, 4 * NB], f32, tag="tcc")
            nc.scalar.activation(tcc[:], c[:], AF.Tanh)
            u = work.tile([128, 4 * NB], f16, tag="u")
            nc.vector.tensor_mul(u[:], sig[:, 8 * NB:], tcc[:])
            hp = ph.tile([PROJ, NB], f32, tag="hp")
            for k in range(4):
                nc.tensor.matmul(hp[:], WRn[:, PROJ * k:PROJ * (k + 1)],
                                 u[:, NB * k:NB * (k + 1)],
                                 start=(k == 0), stop=(k == 3))
            nc.vector.tensor_copy(primary, hp[:])
            for dst in gp_dsts:
                nc.gpsimd.tensor_copy(dst, primary)

        # ---------------- encoder ----------------
        nc.gpsimd.tensor_copy(rhs0[0][0:INF, :], xe[:, 0:NB])
        for t in range(seq + 1):
            b, nb = t % 2, (t + 1) % 2
            if t < seq:
                if t + 1 < seq:
                    nc.gpsimd.tensor_copy(rhs0[nb][0:INF, :],
                                          xe[:, NB * (t + 1):NB * (t + 2)])
                layer_step("e0", rhs0[b], c0, rhs1[b][64:112, :],
                           [rhs0[nb][64:112, :]])
            if t >= 1:
                tp = t - 1
                bp, nbp = tp % 2, (tp + 1) % 2
                layer_step("e1", rhs1[bp], c1, rhs1[nbp][0:48, :],
                           [hs1[:, NB * tp:NB * (tp + 1)]])

        # ---------------- decoder ----------------
        nc.gpsimd.tensor_copy(rhs0d[0][0:48, :], fprev)
        nc.gpsimd.tensor_copy(rhs0d[0][64:112, :], rhs0[seq % 2][64:112, :])
        nc.sync.dma_start(rhs0d[0][112:128, :], d_dxe[:, 0:NB])
        for t in range(fut):
            b, nb = t % 2, (t + 1) % 2
            if t + 1 < fut:
                nc.sync.dma_start(rhs0d[nb][112:128, :],
                                  d_dxe[:, NB * (t + 1):NB * (t + 2)])
            layer_step("d0", rhs0d[b], c0, rhs1[b][64:112, :],
                       [rhs0d[nb][64:112, :]])
            layer_step("d1", rhs1[b], c1, rhs1[nb][0:48, :],
                       [rhs0d[nb][0:48, :], preds[:, NB * t:NB * (t + 1)]])

        # ---------------- theta einsums ----------------
        ones48 = const.tile([PROJ, 1], f32, tag="ones48")
        nc.vector.memset(ones48[:], 1.0)

        def reduce_out(src, thv, n, out_sb, tag):
            # out[0, j] = sum_p thv[p, j%NB] * src[p, j]
            mulbuf = state.tile([PROJ, n], f32, tag=f"mul_{tag}")
            nt = n // NB
            nc.vector.tensor_mul(
                mulbuf[:].rearrange("p (t b) -> p t b", b=NB),
                src[:].rearrange("p (t b) -> p t b", b=NB),
                thv[:, None, :].broadcast_to([PROJ, nt, NB]))
            for j in range(0, n, 512):
                w = min(512, n - j)
                ps = pout.tile([1, 512], f32, tag="pout")
                nc.tensor.matmul(ps[:, :w], ones48[:], mulbuf[:, j:j + w],
                                 start=True, stop=True)
                nc.vector.tensor_copy(out_sb[:, j:j + w], ps[:, :w])

        bct = state.tile([1, seq * NB], f32, tag="bct")
        fct = state.tile([1, fut * NB], f32, tag="fct")
        reduce_out(hs1, thb, seq * NB, bct, "bc")
        reduce_out(preds, thf, fut * NB, fct, "fc")
        nc.sync.dma_start(d_bc[:], bct[:])
        nc.sync.dma_start(d_fc[:], fct[:])

    nc.finalize()
    return nc


_NC_CACHE = {}


def _get_nc(seq, fut):
    key = (seq, fut)
    if key not in _NC_CACHE:
        _NC_CACHE[key] = _build_nc(seq, fut)
    return _NC_CACHE[key]


def kernel(theta, insample_x_t, outsample_x_t, _seq=SEQ, _fut=FUT, **weights):
    from concourse.bass_utils import run_bass_kernel_spmd

    inp = dict(weights)
    inp["theta"] = theta
    inp["insample_x_t"] = insample_x_t
    inp["outsample_x_t"] = outsample_x_t

    nc = _get_nc(_seq, _fut)
    combo = _prep_shared(inp)
    in_maps = [_prep_core(inp, combo, NB * i, _seq, _fut) for i in range(NCORES)]
    res = run_bass_kernel_spmd(nc, in_maps, list(range(NCORES)))

    bc = np.empty((B, _seq), np.float32)
    fc = np.empty((B, _fut), np.float32)
    for i in range(NCORES):
        r = res.results[i]
        bc[NB * i:NB * (i + 1)] = r["bc_t"].reshape(_seq, NB).T
        fc[NB * i:NB * (i + 1)] = r["fc_t"].reshape(_fut, NB).T
    return bc, fc


def bench(inp, _seq=SEQ, _fut=FUT):
    """Profiled run; returns HW exec time in ns (max across cores)."""
    from concourse.bass_utils import run_bass_kernel_spmd

    nc = _get_nc(_seq, _fut)
    combo = _prep_shared(inp)
    in_maps = [_prep_core(inp, combo, NB * i, _seq, _fut) for i in range(NCORES)]
    res = run_bass_kernel_spmd(nc, in_maps, list(range(NCORES)), trace=True)
    t = res.exec_time_ns
    if t is None:
        t = res.mean_exec_time_ns
    return t


# revision 14
# speedup vs baseline: 1.1411x; 1.0230x over previous
"""Trainium2 Bass kernel for nn_ExogenousBasisLSTM.

Strategy (8 NeuronCores, data-parallel over batch: 16 samples/core):
- Everything in "T-layout": gate/hidden dim on SBUF partitions, batch on free.
- Per layer-step, gates^T [128, 16*16] accumulate in PSUM from:
    1 bias matmul  (lhsT = bias chunks [16,128], rhs = const delta [16,256])
    16 chunk matmuls (lhsT = packed weights [K,128], rhs = state block [K,16])
  Input-side GEMMs fold into the chunk matmuls; biases via the bias matmul,
  so rhs tiles contain only data blocks at partition-aligned bases
  (compute-engine APs must start at partition 0/32/64/96):
    enc L0 rhs [112,16]: x(0:64)    | h0(64:112)
    L1     rhs [112,16]: h1(0:48)   | 0 | h0(64:112)
    dec L0 rhs [128,16]: prev(0:48) | 0 | h0(64:112) | exog(112:128 via DMA)
- rhs tiles ping-pong so next-step input copies overlap current step.
- Gate columns reordered [i,f,o,g]: one sigmoid [128,192], one tanh [128,64].
- c stays fp32 [128,64]; matmul operands fp16 (FWL + 1 cycle/row), PSUM fp32.
- h = (sig_o*tanh c) @ Whr^T via 4 accumulating matmuls; h^T [48,16] copied
  (fp16) straight into next-step rhs tiles.
- All fp16 constants arrive in two combo DMAs (PE-consumed / DVE-consumed) so
  no compute instruction ever waits on two DMA-HW queues.
- Final theta einsums on device: broadcast-multiply + ones-reduction matmuls.
"""

from contextlib import ExitStack

import numpy as np

B, INF, OUTF, SEQ, FUT = 128, 64, 16, 512, 96
H, PROJ = 512, 48
NB = 16          # batch per core
NCORES = 8
G4 = 4 * H       # 2048

# gate reorder: old [i,f,g,o] -> new [i,f,o,g]
_PERM = np.concatenate([np.arange(512), 512 + np.arange(512),
                        1536 + np.arange(512), 1024 + np.arange(512)])

_F16 = np.float16

# column offsets inside combo tensors
_PE_SEG = [("W_e0", G4), ("W_e1", G4), ("W_d0", G4), ("W_d1", G4),
           ("WR_e0", 4 * PROJ), ("WR_e1", 4 * PROJ),
           ("WR_d0", 4 * PROJ), ("WR_d1", 4 * PROJ),
           ("sb_e0", 128), ("sb_e1", 128), ("sb_d0", 128), ("sb_d1", 128),
           ("delta", 16 * NB)]


def _pe_offsets():
    off, out = 0, {}
    for name, w in _PE_SEG:
        out[name] = (off, w)
        off += w
    return out, off


def _prep_shared(inp):
    """Pack all PE-side constants into one [128, N] fp16 array."""
    offs, total = _pe_offsets()
    combo = np.zeros((128, total), _F16)

    def put(name, arr):  # arr [rows, w] float32/float16
        o, w = offs[name]
        combo[:arr.shape[0], o:o + w] = arr.astype(_F16)

    for net, l, name in (("enc", 0, "e0"), ("enc", 1, "e1"),
                         ("dec", 0, "d0"), ("dec", 1, "d1")):
        Wih = np.asarray(inp[f"{net}_Wih{l}"])[_PERM]
        Whh = np.asarray(inp[f"{net}_Whh{l}"])[_PERM]
        bias = (np.asarray(inp[f"{net}_bih{l}"]) +
                np.asarray(inp[f"{net}_bhh{l}"]))[_PERM]
        z16 = np.zeros((16, G4), np.float32)
        if name == "e0":
            W = np.concatenate([Wih.T, Whh.T], axis=0)                 # 112
        elif name == "d0":
            W = np.concatenate([Wih[:, :PROJ].T, z16, Whh.T,
                                Wih[:, PROJ:].T], axis=0)              # 128
        else:
            W = np.concatenate([Whh.T, z16, Wih.T], axis=0)            # 112
        put(f"W_{name}", W)
        Whr = np.asarray(inp[f"{net}_Whr{l}"])
        WR = np.zeros((128, 4 * PROJ), np.float32)
        for k in range(4):
            WR[:, PROJ * k:PROJ * (k + 1)] = Whr[:, 128 * k:128 * (k + 1)].T
        put(f"WR_{name}", WR)
        put(f"sb_{name}", bias.reshape(16, 128))

    delta = np.zeros((16, 16 * NB), np.float32)
    for m in range(16):
        delta[m, NB * m:NB * (m + 1)] = 1.0
    put("delta", delta)
    return combo


def _prep_core(inp, combo, s, seq, fut):
    """in_map for the core owning batch slice [s, s+NB)."""
    ins = np.asarray(inp["insample_x_t"])[s:s + NB]      # [NB, 64, 512]
    outs = np.asarray(inp["outsample_x_t"])[s:s + NB]    # [NB, 16, 96]
    theta = np.asarray(inp["theta"])[s:s + NB]

    # dve combo: xe (64r, seq*NB) | fprev (48r, NB) | thb,thf (48r, 2*NB)
    dve = np.zeros((128, seq * NB + 3 * NB), _F16)
    dve[:INF, :seq * NB] = \
        ins[:, :, :seq].transpose(1, 2, 0).reshape(INF, seq * NB)
    dve[:PROJ, seq * NB:seq * NB + NB] = ins[:, :PROJ, SEQ - 1].T  # first_prev
    dve[:PROJ, seq * NB + NB:seq * NB + 2 * NB] = theta[:, PROJ:].T
    dve[:PROJ, seq * NB + 2 * NB:] = theta[:, :PROJ].T
    dxe = outs[:, :, :fut].transpose(1, 2, 0).reshape(OUTF, fut * NB)

    return {"combo_pe": combo, "combo_dve": np.ascontiguousarray(dve),
            "dxe": np.ascontiguousarray(dxe).astype(_F16)}


def _build_nc(seq, fut):
    import concourse.bacc as bacc
    import concourse.mybir as mybir
    import concourse.tile as tile

    f16 = mybir.dt.float16
    f32 = mybir.dt.float32
    f32r = mybir.dt.float32r
    AF = mybir.ActivationFunctionType

    offs, pe_total = _pe_offsets()

    nc = bacc.Bacc()
    d_pe = nc.declare_dram_parameter("combo_pe", [128, pe_total], f16,
                                     isOutput=False)
    d_dve = nc.declare_dram_parameter("combo_dve", [128, seq * NB + 3 * NB], f16,
                                      isOutput=False)
    d_dxe = nc.declare_dram_parameter("dxe", [OUTF, fut * NB], f16,
                                      isOutput=False)
    d_bc = nc.declare_dram_parameter("bc_t", [1, seq * NB], f32, isOutput=True)
    d_fc = nc.declare_dram_parameter("fc_t", [1, fut * NB], f32, isOutput=True)

    with tile.TileContext(nc) as tc, ExitStack() as ctx:
        const = ctx.enter_context(tc.tile_pool(name="const", bufs=1))
        state = ctx.enter_context(tc.tile_pool(name="state", bufs=1))
        work = ctx.enter_context(tc.tile_pool(name="work", bufs=4))
        pgates = ctx.enter_context(tc.tile_pool(name="pgates", bufs=2, space="PSUM"))
        ph = ctx.enter_context(tc.tile_pool(name="ph", bufs=2, space="PSUM"))
        pout = ctx.enter_context(tc.tile_pool(name="pout", bufs=2, space="PSUM"))

        pe = const.tile([128, pe_total], f16, tag="pe")
        nc.sync.dma_start(pe[:], d_pe[:])
        dve = const.tile([128, seq * NB + 3 * NB], f16, tag="dve")
        nc.sync.dma_start(dve[:], d_dve[:])

        def cseg(name, rows=128):
            o, w = offs[name]
            return pe[0:rows, o:o + w]

        xe = dve[0:INF, 0:seq * NB]
        fprev = dve[0:PROJ, seq * NB:seq * NB + NB]
        thb = dve[0:PROJ, seq * NB + NB:seq * NB + 2 * NB]
        thf = dve[0:PROJ, seq * NB + 2 * NB:seq * NB + 3 * NB]

        rhs0, rhs1, rhs0d = [], [], []
        for i in range(2):
            t = state.tile([112, NB], f16, tag=f"rhs0_{i}")
            nc.vector.memset(t[:], 0.0)
            rhs0.append(t)
            t = state.tile([112, NB], f16, tag=f"rhs1_{i}")
            nc.vector.memset(t[:], 0.0)
            rhs1.append(t)
            t = state.tile([128, NB], f16, tag=f"rhs0d_{i}")
            nc.vector.memset(t[:], 0.0)
            rhs0d.append(t)

        c0 = state.tile([128, 4 * NB], f32, tag="c0")
        c1 = state.tile([128, 4 * NB], f32, tag="c1")
        nc.vector.memset(c0[:], 0.0)
        nc.vector.memset(c1[:], 0.0)
        hs1 = state.tile([PROJ, seq * NB], f16, tag="hs1")
        preds = state.tile([PROJ, fut * NB], f16, tag="preds")

        delta = cseg("delta", 16)

        def layer_step(wname, rhs, c, primary, gp_dsts):
            Wn = cseg(f"W_{wname}", rhs.shape[0])
            WRn = cseg(f"WR_{wname}")
            sbias = cseg(f"sb_{wname}", 16)
            # bank B first (o,g chunks 8-15) so tanh_g / sig_o overlap the
            # remaining matmul burst; bank A (i,f chunks 0-7) finishes last
            # and only sig_if sits on the chain.
            gB = pgates.tile([128, 8 * NB], f32, tag="gatesB")
            nc.tensor.matmul(gB[:], sbias, delta[:, 8 * NB:], start=True,
                             stop=False)
            for m in range(8, 16):
                nc.tensor.matmul(gB[:, NB * (m - 8):NB * (m - 7)],
                                 Wn[:, 128 * m:128 * (m + 1)], rhs[:],
                                 start=False, stop=(m == 15))
            gA = pgates.tile([128, 8 * NB], f32, tag="gatesA")
            nc.tensor.matmul(gA[:], sbias, delta[:, :8 * NB], start=True,
                             stop=False)
            for m in range(8):
                nc.tensor.matmul(gA[:, NB * m:NB * (m + 1)],
                                 Wn[:, 128 * m:128 * (m + 1)], rhs[:],
                                 start=False, stop=(m == 7))
            tg = work.tile([128, 4 * NB], f32, tag="tg")
            nc.scalar.activation(tg[:], gB[:, 4 * NB:], AF.Tanh)
            so = work.tile([128, 4 * NB], f32, tag="so")
            nc.scalar.activation(so[:], gB[:, :4 * NB], AF.Sigmoid)
            sif = work.tile([128, 8 * NB], f32, tag="sif")
            nc.scalar.activation(sif[:], gA[:], AF.Sigmoid)
            nc.vector.tensor_mul(c[:], c[:], sif[:, 4 * NB:])
            m1 = work.tile([128, 4 * NB], f32, tag="m1")
            nc.vector.tensor_mul(m1[:], sif[:, :4 * NB], tg[:])
            nc.vector.tensor_add(c[:], c[:], m1[:])
            tcc = work.tile([128, 4 * NB], f32, tag="tcc")
            nc.scalar.activation(tcc[:], c[:], AF.Tanh)
            u = work.tile([128, 4 * NB], f16, tag="u")
            nc.vector.tensor_mul(u[:], so[:], tcc[:])
            hp = ph.tile([PROJ, NB], f32, tag="hp")
            for k in range(4):
                nc.tensor.matmul(hp[:], WRn[:, PROJ * k:PROJ * (k + 1)],
                                 u[:, NB * k:NB * (k + 1)],
                                 start=(k == 0), stop=(k == 3))
            nc.vector.tensor_copy(primary, hp[:])
            for dst in gp_dsts:
                nc.gpsimd.tensor_copy(dst, primary)

        # ---------------- encoder ----------------
        nc.gpsimd.tensor_copy(rhs0[0][0:INF, :], xe[:, 0:NB])
        for t in range(seq + 1):
            b, nb = t % 2, (t + 1) % 2
            if t < seq:
                if t + 1 < seq:
                    nc.gpsimd.tensor_copy(rhs0[nb][0:INF, :],
                                          xe[:, NB * (t + 1):NB * (t + 2)])
                layer_step("e0", rhs0[b], c0, rhs1[b][64:112, :],
                           [rhs0[nb][64:112, :]])
            if t >= 1:
                tp = t - 1
                bp, nbp = tp % 2, (tp + 1) % 2
                layer_step("e1", rhs1[bp], c1, rhs1[nbp][0:48, :],
                           [hs1[:, NB * tp:NB * (tp + 1)]])

        # ---------------- decoder ----------------
        nc.gpsimd.tensor_copy(rhs0d[0][0:48, :], fprev)
        nc.gpsimd.tensor_copy(rhs0d[0][64:112, :], rhs0[seq % 2][64:112, :])
        nc.sync.dma_start(rhs0d[0][112:128, :], d_dxe[:, 0:NB])
        for t in range(fut):
            b, nb = t % 2, (t + 1) % 2
            if t + 1 < fut:
                nc.sync.dma_start(rhs0d[nb][112:128, :],
                                  d_dxe[:, NB * (t + 1):NB * (t + 2)])
            layer_step("d0", rhs0d[b], c0, rhs1[b][64:112, :],
                       [rhs0d[nb][64:112, :]])
            layer_step("d1", rhs1[b], c1, rhs1[nb][0:48, :],
                       [rhs0d[nb][0:48, :], preds[:, NB * t:NB * (t + 1)]])

        # ---------------- theta einsums ----------------
        ones48 = const.tile([PROJ, 1], f32, tag="ones48")
        nc.vector.memset(ones48[:], 1.0)

        def reduce_out(src, thv, n, out_sb, tag):
            # out[0, j] = sum_p thv[p, j%NB] * src[p, j]
            mulbuf = state.tile([PROJ, n], f32, tag=f"mul_{tag}")
            nt = n // NB
            nc.vector.tensor_mul(
                mulbuf[:].rearrange("p (t b) -> p t b", b=NB),
                src[:].rearrange("p (t b) -> p t b", b=NB),
                thv[:, None, :].broadcast_to([PROJ, nt, NB]))
            for j in range(0, n, 512):
                w = min(512, n - j)
                ps = pout.tile([1, 512], f32, tag="pout")
                nc.tensor.matmul(ps[:, :w], ones48[:], mulbuf[:, j:j + w],
                                 start=True, stop=True)
                nc.vector.tensor_copy(out_sb[:, j:j + w], ps[:, :w])

        bct = state.tile([1, seq * NB], f32, tag="bct")
        fct = state.tile([1, fut * NB], f32, tag="fct")
        reduce_out(hs1, thb, seq * NB, bct, "bc")
        reduce_out(preds, thf, fut * NB, fct, "fc")
        nc.sync.dma_start(d_bc[:], bct[:])
        nc.sync.dma_start(d_fc[:], fct[:])

    nc.finalize()
    return nc


_NC_CACHE = {}


def _get_nc(seq, fut):
    key = (seq, fut)
    if key not in _NC_CACHE:
        _NC_CACHE[key] = _build_nc(seq, fut)
    return _NC_CACHE[key]


def kernel(theta, insample_x_t, outsample_x_t, _seq=SEQ, _fut=FUT, **weights):
    from concourse.bass_utils import run_bass_kernel_spmd

    inp = dict(weights)
    inp["theta"] = theta
    inp["insample_x_t"] = insample_x_t
    inp["outsample_x_t"] = outsample_x_t

    nc = _get_nc(_seq, _fut)
    combo = _prep_shared(inp)
    in_maps = [_prep_core(inp, combo, NB * i, _seq, _fut) for i in range(NCORES)]
    res = run_bass_kernel_spmd(nc, in_maps, list(range(NCORES)))

    bc = np.empty((B, _seq), np.float32)
    fc = np.empty((B, _fut), np.float32)
    for i in range(NCORES):
        r = res.results[i]
        bc[NB * i:NB * (i + 1)] = r["bc_t"].reshape(_seq, NB).T
        fc[NB * i:NB * (i + 1)] = r["fc_t"].reshape(_fut, NB).T
    return bc, fc


def bench(inp, _seq=SEQ, _fut=FUT):
    """Profiled run; returns HW exec time in ns (max across cores)."""
    from concourse.bass_utils import run_bass_kernel_spmd

    nc = _get_nc(_seq, _fut)
    combo = _prep_shared(inp)
    in_maps = [_prep_core(inp, combo, NB * i, _seq, _fut) for i in range(NCORES)]
    res = run_bass_kernel_spmd(nc, in_maps, list(range(NCORES)), trace=True)
    t = res.exec_time_ns
    if t is None:
        t = res.mean_exec_time_ns
    return t
